# revision 27
# baseline (speedup 1.0000x reference)
"""Trainium2 Bass kernel: image -> additive-sinusoid audio encoding.

Math (per batch image b):
  gray = 255 * (w . rgb);  rev = flip(gray, rows);  avg = mean(gray)
  px   = clip(3*rev - 2*avg, 0, 255)
  A    = where(px==0, 0, exp(ln10 * (px/16 - 15) / 10))          # [M=64 rows, N=64 cols]
  y[t] = sum_m A[m, col(t)] * sin(W[m]*t*dt + PHI0[m]),  col(t) = min(t//361, 63)
  audio= clip(0.5 + 2048*y, -32768, 32767)                       # [ns=23152]

Kernel strategy: t = n*361 + r  =>  angle = theta[i,n] + beta[i,r] (row flip folded
into the host tables), so  sinmat = sin(theta)cos(beta) + cos(theta)sin(beta) and
the gathered einsum becomes dense fp16 matmuls of P/Q = A*sin(theta)/A*cos(theta)
against constant cos/sin(beta) banks widened to r<409 so the audio tail falls out
of the same matmul. The amplitude transform is factored as
  A = mask(z) * min(z, A255),  z = exp(c*gray + b) * exp(-c*2avg)
so the big exp (Et, on ACT) depends only on the gray image and runs parallel to
the mean reduction; only a [128,2]-sized exp (Ec) sits behind the mean.
Data-parallel over batch: 8 images per NeuronCore, layout [128 partitions =
(batch-half, image-row), free = (b2, col)]. The sinusoid banks carry 2*2048 so
the device emits 2*l in saturating fp16; the host halves and applies the final
clip. Output ships as one dense [128, 409] block per (batch-half, column-half)
(junk r>=361 columns included except n=63 where they ARE the tail), so the
whole store is four trivially-contiguous DMAs.
"""

import os

import numpy as np

# ---- problem constants (from the nn.Module definition; input-independent) ----
M = 64
N = 64
FL, FH, FS, T = 80.0, 7600.0, 22050, 1.05
NS = 2 * int(0.5 * FS * T)  # 23152
NUM = NS // N  # 361
RMAX = NS - (N - 1) * NUM  # 409 (last column's sample count)
DT = float(np.float32(1.0 / FS))  # reference rounds dt to f32 (jnp weak typing)
TWO_PI = 2.0 * np.pi
B = 64
N_CORES = 8
B_LOC = B // N_CORES  # 8 images per core
SCALE2 = 2.0 * (0.5 / np.sqrt(M)) * 32768.0  # 4096: device computes 2*l
LN10 = float(np.log(10.0))
EXP_A = LN10 / 160.0
EXP_B = -1.5 * LN10
W0, W1, W2 = 0.2989, 0.5870, 0.1140
C00 = 3.0 * 255.0 * W0  # fold of the 3*255*w0 scale into the gray accumulator
R1 = W1 / W0
R2 = W2 / W0
KAVG2 = 2.0 * 255.0 * W0 / 4096.0  # sum(t) -> 2*avg(gray255) weighting
A0 = float(10.0 ** (-1.5))  # A at px=0 (mask threshold)
A255 = float(10.0 ** (255.0 / 160.0 - 1.5))  # A at px=255 (upper clamp)
FCLIP = 65504.0  # fp16 max; 2*32767 saturates here, host rescales+clips
DEBUG = os.environ.get("BASS_KERNEL_DEBUG", "0") == "1"

# tabA fp16 [128, 640]: stbc | ctbc | nblk    tabB fp16 [128, 818]: cb | sb
ST0 = 0
CT0 = 256
NB0 = 512
TABAW = 640
CB0 = 0
SB0 = RMAX
TABBW = 2 * RMAX


def _make_tables():
    # LCG phase bank (faithful port, ir starts at 0)
    ia, ic, im = 9301, 49297, 233280
    ir = 0
    phi = []
    for _ in range(M):
        ir = (ir * ia + ic) % im
        phi.append(TWO_PI * ir / im)
    phi32 = np.array(phi, np.float64).astype(np.float32)
    w32 = (TWO_PI * FL * (FH / FL) ** (np.arange(M) / (M - 1))).astype(np.float32)

    # fold the row flip (tf.reverse on axis 1) into the tables: row i uses W[63-i]
    wf = w32[::-1].astype(np.float64)
    phif = phi32[::-1].astype(np.float64)

    n_idx = np.arange(N, dtype=np.float64)
    theta = wf[:, None] * (n_idx[None, :] * NUM * DT) + phif[:, None]  # [64, 64]
    st = np.sin(theta).astype(np.float16)
    ct = np.cos(theta).astype(np.float16)

    r_idx = np.arange(RMAX, dtype=np.float64)
    beta = wf[:, None] * (r_idx[None, :] * DT)  # [64, RMAX]
    cb = (SCALE2 * np.cos(beta)).astype(np.float16)
    sb = (SCALE2 * np.sin(beta)).astype(np.float16)

    tabA = np.zeros((128, TABAW), np.float16)
    tabA[:, ST0 : ST0 + 256] = np.tile(st[None, :, None, :], (2, 1, 4, 1)).reshape(
        128, 256
    )
    tabA[:, CT0 : CT0 + 256] = np.tile(ct[None, :, None, :], (2, 1, 4, 1)).reshape(
        128, 256
    )
    blk = np.zeros((128, 128), np.float64)
    blk[:64, :64] = KAVG2
    blk[64:, 64:] = KAVG2
    tabA[:, NB0 : NB0 + 128] = blk.astype(np.float16)

    tabB = np.zeros((128, TABBW), np.float16)
    tabB[:, CB0 : CB0 + RMAX] = np.tile(cb, (2, 1))
    tabB[:, SB0 : SB0 + RMAX] = np.tile(sb, (2, 1))
    return {"tabA": tabA, "tabB": tabB}


_TABLES = None


def tables():
    global _TABLES
    if _TABLES is None:
        _TABLES = _make_tables()
    return _TABLES


def build_nc():
    import concourse.bacc as bacc
    import concourse.bass as bass
    import concourse.mybir as mybir
    import concourse.tile as tile

    f32 = mybir.dt.float32
    f16 = mybir.dt.float16
    Alu = mybir.AluOpType
    Act = mybir.ActivationFunctionType

    nc = bacc.Bacc(
        "TRN2",
        target_bir_lowering=False,
        debug=False,
        num_devices=N_CORES,
        enable_asserts=False,
    )

    # host ships x fp16 pre-transposed to [bh, i, (b2 j c)] so each b2-pair
    # half is one 3-dim HWDGE AP with contiguous 768B descriptors
    x_d = nc.dram_tensor("x", [2, 64, 768], f16, kind="ExternalInput")
    tabA_d = nc.dram_tensor("tabA", [128, TABAW], f16, kind="ExternalInput")
    tabB_d = nc.dram_tensor("tabB", [128, TABBW], f16, kind="ExternalInput")
    # audio4[g=2bh+s]: [p=(i2,n), r] raw 2*l block; host slices r<361 + tail.
    # One DRAM tensor per DMA (slice-writes of one tensor got WAW-eliminated).
    audio4_d = [
        nc.dram_tensor(f"audio4_{g}", [128, RMAX], f16, kind="ExternalOutput")
        for g in range(4)
    ]

    with tile.TileContext(nc) as tc:
        with (
            tc.tile_pool(name="consts", bufs=1) as consts,
            tc.tile_pool(name="work", bufs=1) as work,
            tc.tile_pool(name="psum_y", bufs=4, space=bass.MemorySpace.PSUM) as psum_y,
            tc.tile_pool(name="psum_m", bufs=2, space=bass.MemorySpace.PSUM) as psum_m,
        ):
            # ---- input image [p=(bh,i), (b2, j, c)] fp16, split by b2-pair,
            # then tabA behind it -- all on the SP ring (FIFO: image first) ----
            X = work.tile([128, 768], f16)
            tabA = consts.tile([128, TABAW], f16)
            tabB = consts.tile([128, TABBW], f16)
            xv = x_d[:].rearrange("bh i f -> (bh i) f")
            for s in range(2):
                fs = slice(384 * s, 384 * (s + 1))
                nc.sync.dma_start(out=X[:, fs], in_=xv[:, fs])

            # ---- tiny consts + tables on Pool; the delay memset holds the
            # table transfers back ~0.6us so X gets the SDMA engines first ----
            expb = consts.tile([128, 1], f32)
            nc.gpsimd.memset(expb, EXP_B)
            delayt = consts.tile([128, 640], f16)
            nc.gpsimd.memset(delayt, 0.0)
            nc.gpsimd.dma_start(out=tabA, in_=tabA_d[:])
            nc.gpsimd.dma_start(out=tabB, in_=tabB_d[:])
            stbc = tabA[:, ST0 : ST0 + 256].rearrange("p (a b) -> p a b", b=64)
            ctbc = tabA[:, CT0 : CT0 + 256].rearrange("p (a b) -> p a b", b=64)
            nblk = tabA[:, NB0 : NB0 + 128]
            cb = tabB[:, CB0 : CB0 + RMAX]
            sb = tabB[:, SB0 : SB0 + RMAX]

            # ---- ACT exp-table preload (off critical path) ----
            escr = consts.tile([128, 1], f32)
            nc.scalar.activation(out=escr, in_=expb, func=Act.Exp, bias=0.0, scale=0.0)

            Xc = X[:].rearrange("p (q c) -> p q c", c=3)
            t1 = work.tile([128, 4, 64], f32)
            t = work.tile([128, 4, 64], f32)
            rs32 = work.tile([128, 4], f32)
            rs16 = work.tile([128, 4], f16)
            Et = work.tile([128, 4, 64], f16)
            Ec = work.tile([128, 4], f16)
            z = work.tile([128, 4, 64], f16)
            mz = work.tile([128, 4, 64], f16)
            A = work.tile([128, 4, 64], f16)
            P = work.tile([128, 256], f16)
            Q = work.tile([128, 256], f16)
            U = work.tile([128, 4, RMAX], f16)
            t1f = t1[:].rearrange("p a b -> p (a b)")
            tf = t[:].rearrange("p a b -> p (a b)")
            Pv = P[:].rearrange("p (a b) -> p a b", b=64)
            Qv = Q[:].rearrange("p (a b) -> p a b", b=64)
            cs_tiles = []

            lowp = nc.allow_low_precision(
                reason="fp16 image chain: x is 8-bit-scale data; quantization "
                "well under the 2e-2 gate (measured ~4-6e-3 end to end)"
            )
            with lowp:
                # ---- gray (f32: the exp amplifies input error); rowsums ride
                # the second gray op's accumulator (one per b2 block) ----
                for s in range(2):
                    b2s = slice(2 * s, 2 * s + 2)
                    q128 = slice(128 * s, 128 * (s + 1))
                    nc.vector.scalar_tensor_tensor(
                        out=t1f[:, q128], in0=Xc[:, q128, 1], scalar=float(R1),
                        in1=Xc[:, q128, 0], op0=Alu.mult, op1=Alu.add,
                    )
                    for b2 in (2 * s, 2 * s + 1):
                        q64 = slice(64 * b2, 64 * (b2 + 1))
                        nc.vector.scalar_tensor_tensor(
                            out=tf[:, q64], in0=Xc[:, q64, 2], scalar=float(R2),
                            in1=t1f[:, q64], op0=Alu.mult, op1=Alu.add,
                            accum_out=rs32[:, b2 : b2 + 1],
                        )
                    nc.vector.tensor_scalar_mul(
                        out=rs16[:, b2s], in0=rs32[:, b2s], scalar1=1.0
                    )
                    # Et = exp(c*C00*t + b) on ACT, parallel to the mean path
                    nc.scalar.activation(
                        out=Et[:, b2s], in_=t[:, b2s], func=Act.Exp,
                        bias=expb, scale=float(EXP_A * C00),
                    )

                # ---- mean matmuls (PE) + per-image exp(-c*2avg) on ACT ----
                for s in range(2):
                    b2s = slice(2 * s, 2 * s + 2)
                    cs = psum_m.tile([128, 2], f32, tag="cs")
                    cs_tiles.append(cs)
                    nc.tensor.matmul(cs, nblk, rs16[:, b2s], start=True, stop=True)
                for s in range(2):
                    b2s = slice(2 * s, 2 * s + 2)
                    nc.scalar.activation(
                        out=Ec[:, b2s], in_=cs_tiles[s], func=Act.Exp,
                        bias=0.0, scale=float(-EXP_A),
                    )

                # ---- per b2-pair stage: A = mask(z)*min(z,A255) -> P/Q ->
                # matmul -> clip (fp16-saturating; ACT bh=0, DVE bh=1) ----
                for s in range(2):
                    b2s = slice(2 * s, 2 * s + 2)
                    q128 = slice(128 * s, 128 * (s + 1))
                    nc.vector.tensor_mul(
                        out=z[:, b2s], in0=Et[:, b2s],
                        in1=Ec[:, b2s].unsqueeze(2).broadcast_to([128, 2, 64]),
                    )
                    nc.vector.scalar_tensor_tensor(
                        out=mz[:, b2s], in0=z[:, b2s], scalar=float(A0),
                        in1=z[:, b2s], op0=Alu.is_gt, op1=Alu.mult,
                    )
                    nc.vector.tensor_scalar_min(
                        out=A[:, b2s], in0=mz[:, b2s], scalar1=float(A255)
                    )
                    nc.vector.tensor_mul(out=Pv[:, b2s], in0=A[:, b2s], in1=stbc[:, b2s])
                    nc.vector.tensor_mul(out=Qv[:, b2s], in0=A[:, b2s], in1=ctbc[:, b2s])
                    for bh in range(2):
                        g = 2 * bh + s
                        prt = slice(64 * bh, 64 * (bh + 1))
                        y_ps = psum_y.tile([128, RMAX], f32, tag="y")
                        nc.tensor.matmul(
                            y_ps, P[prt, q128], cb[prt, :], start=True, stop=False
                        )
                        nc.tensor.matmul(
                            y_ps, Q[prt, q128], sb[prt, :], start=False, stop=True
                        )
                        # clip: ACT copies bh=0 groups, DVE clips bh=1;
                        # out-DMAs: bh0 on SP, bh1 on ACT
                        if bh == 0:
                            nc.scalar.activation(
                                out=U[:, g], in_=y_ps, func=Act.Copy,
                                bias=0.0, scale=1.0,
                            )
                            nc.sync.dma_start(out=audio4_d[g][:], in_=U[:, g])
                        else:
                            nc.vector.tensor_scalar(
                                out=U[:, g], in0=y_ps,
                                scalar1=-FCLIP, scalar2=FCLIP,
                                op0=Alu.max, op1=Alu.min,
                            )
                            nc.scalar.dma_start(out=audio4_d[g][:], in_=U[:, g])

    nc.compile()
    return nc


_NC = None


def _get_nc():
    global _NC
    if _NC is None:
        _NC = build_nc()
    return _NC


LAST_RESULTS = None


def kernel(x: np.ndarray) -> np.ndarray:
    from concourse.bass_utils import run_bass_kernel_spmd

    x = np.asarray(x, dtype=np.float32)
    assert x.shape == (B, 64, 64, 3), x.shape
    # [B,64,64,3] -> per-core [bh, i, b2, j, c] flattened to [2, 64, 768] fp16
    xr = x.reshape(N_CORES, 2, 4, 64, 64, 3).transpose(0, 1, 3, 2, 4, 5)
    xr = np.ascontiguousarray(xr.reshape(N_CORES, 2, 64, 768).astype(np.float16))

    nc = _get_nc()
    tbl = tables()
    in_maps = []
    for c in range(N_CORES):
        m = {"x": xr[c]}
        m.update(tbl)
        in_maps.append(m)

    trace = os.environ.get("BASS_KERNEL_TRACE", "0") == "1"
    res = run_bass_kernel_spmd(
        nc, in_maps, core_ids=list(range(N_CORES)), trace=trace
    )
    global LAST_RESULTS
    LAST_RESULTS = res
    outs = []
    for r in res.results:
        # audio4[g=2bh+s]: [p=(i2,n), r] holding 2*l; image b = 4bh + 2s + i2
        a = np.empty((B_LOC, NS), np.float32)
        for g in range(4):
            bh, s = divmod(g, 2)
            a4 = r[f"audio4_{g}"].astype(np.float32).reshape(2, 64, RMAX)
            for i2 in range(2):
                b = 4 * bh + 2 * s + i2
                a[b, : N * NUM] = a4[i2, :, :NUM].reshape(-1)
                a[b, N * NUM :] = a4[i2, 63, NUM:RMAX]
        a = np.clip(a * 0.5, -32768.0, 32767.0)
        outs.append(a)
    return np.concatenate(outs, axis=0)


# revision 28
# speedup vs baseline: 1.0341x; 1.0341x over previous
"""Trainium2 Bass kernel: image -> additive-sinusoid audio encoding.

Math (per batch image b):
  gray = 255 * (w . rgb);  rev = flip(gray, rows);  avg = mean(gray)
  px   = clip(3*rev - 2*avg, 0, 255)
  A    = where(px==0, 0, exp(ln10 * (px/16 - 15) / 10))          # [M=64 rows, N=64 cols]
  y[t] = sum_m A[m, col(t)] * sin(W[m]*t*dt + PHI0[m]),  col(t) = min(t//361, 63)
  audio= clip(0.5 + 2048*y, -32768, 32767)                       # [ns=23152]

Kernel strategy: t = n*361 + r  =>  angle = theta[i,n] + beta[i,r] (row flip folded
into the host tables), so  sinmat = sin(theta)cos(beta) + cos(theta)sin(beta) and
the gathered einsum becomes dense fp16 matmuls of P/Q = A*sin(theta)/A*cos(theta)
against constant cos/sin(beta) banks widened to r<409 so the audio tail falls out
of the same matmul. The amplitude transform is factored as
  A = mask(z) * min(z, A255),  z = exp(c*gray + b) * exp(-c*2avg)
so the big exp (Et, on ACT) depends only on the gray image and runs parallel to
the mean reduction; only a [128,2]-sized exp (Ec) sits behind the mean.
Data-parallel over batch: 8 images per NeuronCore, layout [128 partitions =
(batch-half, image-row), free = (b2, col)]. The sinusoid banks carry 2*2048 so
the device emits 2*l in saturating fp16; the host halves and applies the final
clip. Output ships as one dense [128, 409] block per (batch-half, column-half)
(junk r>=361 columns included except n=63 where they ARE the tail), so the
whole store is four trivially-contiguous DMAs.
"""

import os

import numpy as np

# ---- problem constants (from the nn.Module definition; input-independent) ----
M = 64
N = 64
FL, FH, FS, T = 80.0, 7600.0, 22050, 1.05
NS = 2 * int(0.5 * FS * T)  # 23152
NUM = NS // N  # 361
RMAX = NS - (N - 1) * NUM  # 409 (last column's sample count)
DT = float(np.float32(1.0 / FS))  # reference rounds dt to f32 (jnp weak typing)
TWO_PI = 2.0 * np.pi
B = 64
N_CORES = 8
B_LOC = B // N_CORES  # 8 images per core
SCALE2 = 2.0 * (0.5 / np.sqrt(M)) * 32768.0  # 4096: device computes 2*l
LN10 = float(np.log(10.0))
EXP_A = LN10 / 160.0
EXP_B = -1.5 * LN10
W0, W1, W2 = 0.2989, 0.5870, 0.1140
C00 = 3.0 * 255.0 * W0  # fold of the 3*255*w0 scale into the gray accumulator
R1 = W1 / W0
R2 = W2 / W0
KAVG2 = 2.0 * 255.0 * W0 / 4096.0  # sum(t) -> 2*avg(gray255) weighting
A0 = float(10.0 ** (-1.5))  # A at px=0 (mask threshold)
A255 = float(10.0 ** (255.0 / 160.0 - 1.5))  # A at px=255 (upper clamp)
FCLIP = 65504.0  # fp16 max; 2*32767 saturates here, host rescales+clips
DEBUG = os.environ.get("BASS_KERNEL_DEBUG", "0") == "1"

# tabA fp16 [128, 640]: stbc | ctbc | nblk    tabB fp16 [128, 818]: cb | sb
ST0 = 0
CT0 = 256
NB0 = 512
TABAW = 640
CB0 = 0
SB0 = RMAX
TABBW = 2 * RMAX


def _make_tables():
    # LCG phase bank (faithful port, ir starts at 0)
    ia, ic, im = 9301, 49297, 233280
    ir = 0
    phi = []
    for _ in range(M):
        ir = (ir * ia + ic) % im
        phi.append(TWO_PI * ir / im)
    phi32 = np.array(phi, np.float64).astype(np.float32)
    w32 = (TWO_PI * FL * (FH / FL) ** (np.arange(M) / (M - 1))).astype(np.float32)

    # fold the row flip (tf.reverse on axis 1) into the tables: row i uses W[63-i]
    wf = w32[::-1].astype(np.float64)
    phif = phi32[::-1].astype(np.float64)

    n_idx = np.arange(N, dtype=np.float64)
    theta = wf[:, None] * (n_idx[None, :] * NUM * DT) + phif[:, None]  # [64, 64]
    st = np.sin(theta).astype(np.float16)
    ct = np.cos(theta).astype(np.float16)

    r_idx = np.arange(RMAX, dtype=np.float64)
    beta = wf[:, None] * (r_idx[None, :] * DT)  # [64, RMAX]
    cb = (SCALE2 * np.cos(beta)).astype(np.float16)
    sb = (SCALE2 * np.sin(beta)).astype(np.float16)

    tabA = np.zeros((128, TABAW), np.float16)
    tabA[:, ST0 : ST0 + 256] = np.tile(st[None, :, None, :], (2, 1, 4, 1)).reshape(
        128, 256
    )
    tabA[:, CT0 : CT0 + 256] = np.tile(ct[None, :, None, :], (2, 1, 4, 1)).reshape(
        128, 256
    )
    blk = np.zeros((128, 128), np.float64)
    blk[:64, :64] = KAVG2
    blk[64:, 64:] = KAVG2
    tabA[:, NB0 : NB0 + 128] = blk.astype(np.float16)

    tabB = np.zeros((128, TABBW), np.float16)
    tabB[:, CB0 : CB0 + RMAX] = np.tile(cb, (2, 1))
    tabB[:, SB0 : SB0 + RMAX] = np.tile(sb, (2, 1))
    return {"tabA": tabA, "tabB": tabB}


_TABLES = None


def tables():
    global _TABLES
    if _TABLES is None:
        _TABLES = _make_tables()
    return _TABLES


def build_nc():
    import concourse.bacc as bacc
    import concourse.bass as bass
    import concourse.mybir as mybir
    import concourse.tile as tile

    f32 = mybir.dt.float32
    f16 = mybir.dt.float16
    Alu = mybir.AluOpType
    Act = mybir.ActivationFunctionType

    nc = bacc.Bacc(
        "TRN2",
        target_bir_lowering=False,
        debug=False,
        num_devices=N_CORES,
        enable_asserts=False,
    )

    # host ships x fp16 pre-transposed to [bh, i, (b2 j c)] so each b2-pair
    # half is one 3-dim HWDGE AP with contiguous 768B descriptors
    x_d = nc.dram_tensor("x", [2, 64, 768], f16, kind="ExternalInput")
    tabA_d = nc.dram_tensor("tabA", [128, TABAW], f16, kind="ExternalInput")
    tabB_d = nc.dram_tensor("tabB", [128, TABBW], f16, kind="ExternalInput")
    # audio4[g=2bh+s]: [p=(i2,n), r] raw 2*l block; host slices r<361 + tail.
    # One DRAM tensor per DMA (slice-writes of one tensor got WAW-eliminated).
    audio4_d = [
        nc.dram_tensor(f"audio4_{g}", [128, RMAX], f16, kind="ExternalOutput")
        for g in range(4)
    ]

    with tile.TileContext(nc) as tc:
        with (
            tc.tile_pool(name="consts", bufs=1) as consts,
            tc.tile_pool(name="work", bufs=1) as work,
            tc.tile_pool(name="psum_y", bufs=4, space=bass.MemorySpace.PSUM) as psum_y,
            tc.tile_pool(name="psum_m", bufs=2, space=bass.MemorySpace.PSUM) as psum_m,
        ):
            # ---- input image [p=(bh,i), (b2, j, c)] fp16, split by b2-pair,
            # then tabA behind it -- all on the SP ring (FIFO: image first) ----
            X = work.tile([128, 768], f16)
            tabA = consts.tile([128, TABAW], f16)
            tabB = consts.tile([128, TABBW], f16)
            xv = x_d[:].rearrange("bh i f -> (bh i) f")
            nc.sync.dma_start(out=X, in_=xv)

            # ---- tiny consts + tables on Pool; the delay memset holds the
            # table transfers back ~0.6us so X gets the SDMA engines first ----
            expb = consts.tile([128, 1], f32)
            nc.gpsimd.memset(expb, EXP_B)
            delayt = consts.tile([128, 640], f16)
            nc.gpsimd.memset(delayt, 0.0)
            nc.gpsimd.dma_start(out=tabA, in_=tabA_d[:])
            nc.gpsimd.dma_start(out=tabB, in_=tabB_d[:])
            stbc = tabA[:, ST0 : ST0 + 256].rearrange("p (a b) -> p a b", b=64)
            ctbc = tabA[:, CT0 : CT0 + 256].rearrange("p (a b) -> p a b", b=64)
            nblk = tabA[:, NB0 : NB0 + 128]
            cb = tabB[:, CB0 : CB0 + RMAX]
            sb = tabB[:, SB0 : SB0 + RMAX]

            # ---- ACT exp-table preload (off critical path) ----
            escr = consts.tile([128, 1], f32)
            nc.scalar.activation(out=escr, in_=expb, func=Act.Exp, bias=0.0, scale=0.0)

            Xc = X[:].rearrange("p (q c) -> p q c", c=3)
            t1 = work.tile([128, 4, 64], f32)
            t = work.tile([128, 4, 64], f32)
            rs32 = work.tile([128, 4], f32)
            rs16 = work.tile([128, 4], f16)
            Et = work.tile([128, 4, 64], f16)
            Ec = work.tile([128, 4], f16)
            z = work.tile([128, 4, 64], f16)
            mz = work.tile([128, 4, 64], f16)
            A = work.tile([128, 4, 64], f16)
            P = work.tile([128, 256], f16)
            Q = work.tile([128, 256], f16)
            U = work.tile([128, 4, RMAX], f16)
            t1f = t1[:].rearrange("p a b -> p (a b)")
            tf = t[:].rearrange("p a b -> p (a b)")
            Pv = P[:].rearrange("p (a b) -> p a b", b=64)
            Qv = Q[:].rearrange("p (a b) -> p a b", b=64)
            cs_tiles = []

            lowp = nc.allow_low_precision(
                reason="fp16 image chain: x is 8-bit-scale data; quantization "
                "well under the 2e-2 gate (measured ~4-6e-3 end to end)"
            )
            with lowp:
                # ---- gray (f32: the exp amplifies input error); rowsums ride
                # the second gray op's accumulator (one per b2 block) ----
                for s in range(2):
                    b2s = slice(2 * s, 2 * s + 2)
                    q128 = slice(128 * s, 128 * (s + 1))
                    nc.vector.scalar_tensor_tensor(
                        out=t1f[:, q128], in0=Xc[:, q128, 1], scalar=float(R1),
                        in1=Xc[:, q128, 0], op0=Alu.mult, op1=Alu.add,
                    )
                    for b2 in (2 * s, 2 * s + 1):
                        q64 = slice(64 * b2, 64 * (b2 + 1))
                        nc.vector.scalar_tensor_tensor(
                            out=tf[:, q64], in0=Xc[:, q64, 2], scalar=float(R2),
                            in1=t1f[:, q64], op0=Alu.mult, op1=Alu.add,
                            accum_out=rs32[:, b2 : b2 + 1],
                        )
                    nc.vector.tensor_scalar_mul(
                        out=rs16[:, b2s], in0=rs32[:, b2s], scalar1=1.0
                    )
                    # Et = exp(c*C00*t + b) on ACT, parallel to the mean path
                    nc.scalar.activation(
                        out=Et[:, b2s], in_=t[:, b2s], func=Act.Exp,
                        bias=expb, scale=float(EXP_A * C00),
                    )

                # ---- mean matmuls (PE) + per-image exp(-c*2avg) on ACT ----
                for s in range(2):
                    b2s = slice(2 * s, 2 * s + 2)
                    cs = psum_m.tile([128, 2], f32, tag="cs")
                    cs_tiles.append(cs)
                    nc.tensor.matmul(cs, nblk, rs16[:, b2s], start=True, stop=True)
                for s in range(2):
                    b2s = slice(2 * s, 2 * s + 2)
                    nc.scalar.activation(
                        out=Ec[:, b2s], in_=cs_tiles[s], func=Act.Exp,
                        bias=0.0, scale=float(-EXP_A),
                    )

                # ---- per b2-pair stage: A = mask(z)*min(z,A255) -> P/Q ->
                # matmul -> clip (fp16-saturating; ACT bh=0, DVE bh=1) ----
                for s in range(2):
                    b2s = slice(2 * s, 2 * s + 2)
                    q128 = slice(128 * s, 128 * (s + 1))
                    nc.vector.tensor_mul(
                        out=z[:, b2s], in0=Et[:, b2s],
                        in1=Ec[:, b2s].unsqueeze(2).broadcast_to([128, 2, 64]),
                    )
                    nc.vector.scalar_tensor_tensor(
                        out=mz[:, b2s], in0=z[:, b2s], scalar=float(A0),
                        in1=z[:, b2s], op0=Alu.is_gt, op1=Alu.mult,
                    )
                    nc.vector.tensor_scalar_min(
                        out=A[:, b2s], in0=mz[:, b2s], scalar1=float(A255)
                    )
                    nc.vector.tensor_mul(out=Pv[:, b2s], in0=A[:, b2s], in1=stbc[:, b2s])
                    nc.vector.tensor_mul(out=Qv[:, b2s], in0=A[:, b2s], in1=ctbc[:, b2s])
                    for bh in range(2):
                        g = 2 * bh + s
                        prt = slice(64 * bh, 64 * (bh + 1))
                        y_ps = psum_y.tile([128, RMAX], f32, tag="y")
                        nc.tensor.matmul(
                            y_ps, P[prt, q128], cb[prt, :], start=True, stop=False
                        )
                        nc.tensor.matmul(
                            y_ps, Q[prt, q128], sb[prt, :], start=False, stop=True
                        )
                        # clip: ACT copies bh=0 groups, DVE clips bh=1;
                        # out-DMAs: bh0 on SP, bh1 on ACT
                        if bh == 0:
                            nc.scalar.activation(
                                out=U[:, g], in_=y_ps, func=Act.Copy,
                                bias=0.0, scale=1.0,
                            )
                            nc.sync.dma_start(out=audio4_d[g][:], in_=U[:, g])
                        else:
                            nc.vector.tensor_scalar(
                                out=U[:, g], in0=y_ps,
                                scalar1=-FCLIP, scalar2=FCLIP,
                                op0=Alu.max, op1=Alu.min,
                            )
                            nc.scalar.dma_start(out=audio4_d[g][:], in_=U[:, g])

    nc.compile()
    return nc


_NC = None


def _get_nc():
    global _NC
    if _NC is None:
        _NC = build_nc()
    return _NC


LAST_RESULTS = None


def kernel(x: np.ndarray) -> np.ndarray:
    from concourse.bass_utils import run_bass_kernel_spmd

    x = np.asarray(x, dtype=np.float32)
    assert x.shape == (B, 64, 64, 3), x.shape
    # [B,64,64,3] -> per-core [bh, i, b2, j, c] flattened to [2, 64, 768] fp16
    xr = x.reshape(N_CORES, 2, 4, 64, 64, 3).transpose(0, 1, 3, 2, 4, 5)
    xr = np.ascontiguousarray(xr.reshape(N_CORES, 2, 64, 768).astype(np.float16))

    nc = _get_nc()
    tbl = tables()
    in_maps = []
    for c in range(N_CORES):
        m = {"x": xr[c]}
        m.update(tbl)
        in_maps.append(m)

    trace = os.environ.get("BASS_KERNEL_TRACE", "0") == "1"
    res = run_bass_kernel_spmd(
        nc, in_maps, core_ids=list(range(N_CORES)), trace=trace
    )
    global LAST_RESULTS
    LAST_RESULTS = res
    outs = []
    for r in res.results:
        # audio4[g=2bh+s]: [p=(i2,n), r] holding 2*l; image b = 4bh + 2s + i2
        a = np.empty((B_LOC, NS), np.float32)
        for g in range(4):
            bh, s = divmod(g, 2)
            a4 = r[f"audio4_{g}"].astype(np.float32).reshape(2, 64, RMAX)
            for i2 in range(2):
                b = 4 * bh + 2 * s + i2
                a[b, : N * NUM] = a4[i2, :, :NUM].reshape(-1)
                a[b, N * NUM :] = a4[i2, 63, NUM:RMAX]
        a = np.clip(a * 0.5, -32768.0, 32767.0)
        outs.append(a)
    return np.concatenate(outs, axis=0)


# revision 32
# speedup vs baseline: 1.1195x; 1.0826x over previous
"""Trainium2 Bass kernel: image -> additive-sinusoid audio encoding.

Math (per batch image b):
  gray = 255 * (w . rgb);  rev = flip(gray, rows);  avg = mean(gray)
  px   = clip(3*rev - 2*avg, 0, 255)
  A    = where(px==0, 0, exp(ln10 * (px/16 - 15) / 10))          # [M=64 rows, N=64 cols]
  y[t] = sum_m A[m, col(t)] * sin(W[m]*t*dt + PHI0[m]),  col(t) = min(t//361, 63)
  audio= clip(0.5 + 2048*y, -32768, 32767)                       # [ns=23152]

Kernel strategy: t = n*361 + r  =>  angle = theta[i,n] + beta[i,r] (row flip folded
into the host tables), so  sinmat = sin(theta)cos(beta) + cos(theta)sin(beta) and
the gathered einsum becomes dense fp16 matmuls of P/Q = A*sin(theta)/A*cos(theta)
against constant cos/sin(beta) banks widened to r<409 so the audio tail falls out
of the same matmul. The amplitude transform is factored as
  A = mask(z) * min(z, A255),  z = exp(c*gray + b) * exp(-c*2avg)
so the big exp (Et, on ACT) depends only on the gray image and runs parallel to
the mean reduction; only a [128,2]-sized exp (Ec) sits behind the mean.
Data-parallel over batch: 8 images per NeuronCore, layout [128 partitions =
(batch-half, image-row), free = (b2, col)]. The sinusoid banks carry 2*2048 so
the device emits 2*l in saturating fp16; the host halves and applies the final
clip. Output ships as one dense [128, 409] block per (batch-half, column-half)
(junk r>=361 columns included except n=63 where they ARE the tail), so the
whole store is four trivially-contiguous DMAs.
"""

import os

import numpy as np

# ---- problem constants (from the nn.Module definition; input-independent) ----
M = 64
N = 64
FL, FH, FS, T = 80.0, 7600.0, 22050, 1.05
NS = 2 * int(0.5 * FS * T)  # 23152
NUM = NS // N  # 361
RMAX = NS - (N - 1) * NUM  # 409 (last column's sample count)
DT = float(np.float32(1.0 / FS))  # reference rounds dt to f32 (jnp weak typing)
TWO_PI = 2.0 * np.pi
B = 64
N_CORES = 8
B_LOC = B // N_CORES  # 8 images per core
SCALE2 = 2.0 * (0.5 / np.sqrt(M)) * 32768.0  # 4096: device computes 2*l
LN10 = float(np.log(10.0))
EXP_A = LN10 / 160.0
EXP_B = -1.5 * LN10
W0, W1, W2 = 0.2989, 0.5870, 0.1140
C00 = 3.0 * 255.0 * W0  # fold of the 3*255*w0 scale into the gray accumulator
R1 = W1 / W0
R2 = W2 / W0
KAVG2 = 2.0 * 255.0 * W0 / 4096.0  # sum(t) -> 2*avg(gray255) weighting
A0 = float(10.0 ** (-1.5))  # A at px=0 (mask threshold)
A255 = float(10.0 ** (255.0 / 160.0 - 1.5))  # A at px=255 (upper clamp)
FCLIP = 65504.0  # fp16 max; 2*32767 saturates here, host rescales+clips

# tabA fp16 [128, 640]: stbc | ctbc | nblk    tabB fp16 [128, 818]: cb | sb
ST0 = 0
CT0 = 256
NB0 = 512
TABAW = 640
CB0 = 0
SB0 = RMAX
TABBW = 2 * RMAX


def _make_tables():
    # LCG phase bank (faithful port, ir starts at 0)
    ia, ic, im = 9301, 49297, 233280
    ir = 0
    phi = []
    for _ in range(M):
        ir = (ir * ia + ic) % im
        phi.append(TWO_PI * ir / im)
    phi32 = np.array(phi, np.float64).astype(np.float32)
    w32 = (TWO_PI * FL * (FH / FL) ** (np.arange(M) / (M - 1))).astype(np.float32)

    # fold the row flip (tf.reverse on axis 1) into the tables: row i uses W[63-i]
    wf = w32[::-1].astype(np.float64)
    phif = phi32[::-1].astype(np.float64)

    n_idx = np.arange(N, dtype=np.float64)
    theta = wf[:, None] * (n_idx[None, :] * NUM * DT) + phif[:, None]  # [64, 64]
    st = np.sin(theta).astype(np.float16)
    ct = np.cos(theta).astype(np.float16)

    r_idx = np.arange(RMAX, dtype=np.float64)
    beta = wf[:, None] * (r_idx[None, :] * DT)  # [64, RMAX]
    cb = (SCALE2 * np.cos(beta)).astype(np.float16)
    sb = (SCALE2 * np.sin(beta)).astype(np.float16)

    tabA = np.zeros((128, TABAW), np.float16)
    tabA[:, ST0 : ST0 + 256] = np.tile(st[None, :, None, :], (2, 1, 4, 1)).reshape(
        128, 256
    )
    tabA[:, CT0 : CT0 + 256] = np.tile(ct[None, :, None, :], (2, 1, 4, 1)).reshape(
        128, 256
    )
    blk = np.zeros((128, 128), np.float64)
    blk[:64, :64] = KAVG2
    blk[64:, 64:] = KAVG2
    tabA[:, NB0 : NB0 + 128] = blk.astype(np.float16)

    tabB = np.zeros((128, TABBW), np.float16)
    tabB[:, CB0 : CB0 + RMAX] = np.tile(cb, (2, 1))
    tabB[:, SB0 : SB0 + RMAX] = np.tile(sb, (2, 1))
    return {"tabA": tabA, "tabB": tabB}


_TABLES = None


def tables():
    global _TABLES
    if _TABLES is None:
        _TABLES = _make_tables()
    return _TABLES


def build_nc():
    import concourse.bacc as bacc
    import concourse.bass as bass
    import concourse.mybir as mybir
    import concourse.tile as tile

    f32 = mybir.dt.float32
    f16 = mybir.dt.float16
    Alu = mybir.AluOpType
    Act = mybir.ActivationFunctionType

    nc = bacc.Bacc(
        "TRN2",
        target_bir_lowering=False,
        debug=False,
        num_devices=N_CORES,
        enable_asserts=False,
    )

    # host ships x fp16 pre-transposed to [bh, i, (b2 j c)] so each b2-pair
    # half is one 3-dim HWDGE AP with contiguous 768B descriptors
    x_d = nc.dram_tensor("x", [2, 64, 768], f16, kind="ExternalInput")
    tabA_d = nc.dram_tensor("tabA", [128, TABAW], f16, kind="ExternalInput")
    tabB_d = nc.dram_tensor("tabB", [128, TABBW], f16, kind="ExternalInput")
    # audio4[g=2bh+s]: [p=(i2,n), r] raw 2*l block; host slices r<361 + tail.
    # One DRAM tensor per DMA (slice-writes of one tensor got WAW-eliminated).
    audio4_d = [
        nc.dram_tensor(f"audio4_{g}", [128, RMAX], f16, kind="ExternalOutput")
        for g in range(4)
    ]

    with tile.TileContext(nc) as tc:
        with (
            tc.tile_pool(name="consts", bufs=1) as consts,
            tc.tile_pool(name="work", bufs=1) as work,
            tc.tile_pool(name="psum_y", bufs=4, space=bass.MemorySpace.PSUM) as psum_y,
            tc.tile_pool(name="psum_m", bufs=2, space=bass.MemorySpace.PSUM) as psum_m,
        ):
            # ---- input image [p=(bh,i), (b2, j, c)] fp16, split by b2-pair,
            # then tabA behind it -- all on the SP ring (FIFO: image first) ----
            X = work.tile([128, 768], f16)
            tabA = consts.tile([128, TABAW], f16)
            tabB = consts.tile([128, TABBW], f16)
            xv = x_d[:].rearrange("bh i f -> (bh i) f")
            nc.sync.dma_start(out=X, in_=xv)

            # ---- tiny consts + tables on Pool; the delay memset holds the
            # table transfers back ~0.6us so X gets the SDMA engines first ----
            expb = consts.tile([128, 1], f32)
            nc.gpsimd.memset(expb, EXP_B)
            delayt = consts.tile([128, 640], f16)
            nc.gpsimd.memset(delayt, 0.0)
            nc.gpsimd.dma_start(out=tabA, in_=tabA_d[:])
            nc.gpsimd.dma_start(out=tabB, in_=tabB_d[:])
            stbc = tabA[:, ST0 : ST0 + 256].rearrange("p (a b) -> p a b", b=64)
            ctbc = tabA[:, CT0 : CT0 + 256].rearrange("p (a b) -> p a b", b=64)
            nblk = tabA[:, NB0 : NB0 + 128]
            cb = tabB[:, CB0 : CB0 + RMAX]
            sb = tabB[:, SB0 : SB0 + RMAX]

            # ---- ACT exp-table preload (off critical path) ----
            escr = consts.tile([128, 1], f32)
            nc.scalar.activation(out=escr, in_=expb, func=Act.Exp, bias=0.0, scale=0.0)

            Xc = X[:].rearrange("p (q c) -> p q c", c=3)
            t1 = work.tile([128, 4, 64], f32)
            t = work.tile([128, 4, 64], f32)
            rs16 = work.tile([128, 4], f16)
            Et = work.tile([128, 4, 64], f16)
            Ec = work.tile([128, 4], f16)
            z = work.tile([128, 4, 64], f16)
            mz = work.tile([128, 4, 64], f16)
            A = work.tile([128, 4, 64], f16)
            P = work.tile([128, 256], f16)
            Q = work.tile([128, 256], f16)
            U = work.tile([128, 4, RMAX], f16)
            t1f = t1[:].rearrange("p a b -> p (a b)")
            tf = t[:].rearrange("p a b -> p (a b)")
            Pv = P[:].rearrange("p (a b) -> p a b", b=64)
            Qv = Q[:].rearrange("p (a b) -> p a b", b=64)
            cs_tiles = []

            lowp = nc.allow_low_precision(
                reason="fp16 image chain: x is 8-bit-scale data; quantization "
                "well under the 2e-2 gate (measured ~4-6e-3 end to end)"
            )
            with lowp:
                # ---- gray (f32: the exp amplifies input error); rowsums ride
                # the second gray op's accumulator (one per b2 block) ----
                for s in range(2):
                    b2s = slice(2 * s, 2 * s + 2)
                    q128 = slice(128 * s, 128 * (s + 1))
                    nc.vector.scalar_tensor_tensor(
                        out=t1f[:, q128], in0=Xc[:, q128, 1], scalar=float(R1),
                        in1=Xc[:, q128, 0], op0=Alu.mult, op1=Alu.add,
                    )
                    for b2 in (2 * s, 2 * s + 1):
                        q64 = slice(64 * b2, 64 * (b2 + 1))
                        nc.vector.scalar_tensor_tensor(
                            out=tf[:, q64], in0=Xc[:, q64, 2], scalar=float(R2),
                            in1=t1f[:, q64], op0=Alu.mult, op1=Alu.add,
                            accum_out=rs16[:, b2 : b2 + 1],
                        )
                    # Et = exp(c*C00*t + b) on ACT, parallel to the mean path
                    nc.scalar.activation(
                        out=Et[:, b2s], in_=t[:, b2s], func=Act.Exp,
                        bias=expb, scale=float(EXP_A * C00),
                    )

                # ---- mean matmuls (PE) + per-image exp(-c*2avg) on ACT ----
                for s in range(2):
                    b2s = slice(2 * s, 2 * s + 2)
                    cs = psum_m.tile([128, 2], f32, tag="cs")
                    cs_tiles.append(cs)
                    nc.tensor.matmul(cs, nblk, rs16[:, b2s], start=True, stop=True)
                for s in range(2):
                    b2s = slice(2 * s, 2 * s + 2)
                    nc.scalar.activation(
                        out=Ec[:, b2s], in_=cs_tiles[s], func=Act.Exp,
                        bias=0.0, scale=float(-EXP_A),
                    )

                # ---- per b2-pair stage: A = mask(z)*min(z,A255) -> P/Q ->
                # matmul -> clip (fp16-saturating; ACT bh=0, DVE bh=1) ----
                for s in range(2):
                    b2s = slice(2 * s, 2 * s + 2)
                    q128 = slice(128 * s, 128 * (s + 1))
                    nc.vector.tensor_mul(
                        out=z[:, b2s], in0=Et[:, b2s],
                        in1=Ec[:, b2s].unsqueeze(2).broadcast_to([128, 2, 64]),
                    )
                    nc.vector.scalar_tensor_tensor(
                        out=mz[:, b2s], in0=z[:, b2s], scalar=float(A0),
                        in1=z[:, b2s], op0=Alu.is_gt, op1=Alu.mult,
                    )
                    nc.vector.tensor_scalar_min(
                        out=A[:, b2s], in0=mz[:, b2s], scalar1=float(A255)
                    )
                    nc.vector.tensor_mul(out=Pv[:, b2s], in0=A[:, b2s], in1=stbc[:, b2s])
                    nc.vector.tensor_mul(out=Qv[:, b2s], in0=A[:, b2s], in1=ctbc[:, b2s])
                    for bh in range(2):
                        g = 2 * bh + s
                        prt = slice(64 * bh, 64 * (bh + 1))
                        y_ps = psum_y.tile([128, RMAX], f32, tag="y")
                        nc.tensor.matmul(
                            y_ps, P[prt, q128], cb[prt, :], start=True, stop=False
                        )
                        nc.tensor.matmul(
                            y_ps, Q[prt, q128], sb[prt, :], start=False, stop=True
                        )
                        # clip: ACT copies bh=0 groups, DVE clips bh=1;
                        # out-DMAs: bh0 on SP, bh1 on ACT
                        if bh == 0:
                            nc.scalar.activation(
                                out=U[:, g], in_=y_ps, func=Act.Copy,
                                bias=0.0, scale=1.0,
                            )
                            nc.sync.dma_start(out=audio4_d[g][:], in_=U[:, g])
                        else:
                            nc.vector.tensor_scalar(
                                out=U[:, g], in0=y_ps,
                                scalar1=-FCLIP, scalar2=FCLIP,
                                op0=Alu.max, op1=Alu.min,
                            )
                            nc.scalar.dma_start(out=audio4_d[g][:], in_=U[:, g])

    nc.compile()
    return nc


_NC = None


def _get_nc():
    global _NC
    if _NC is None:
        _NC = build_nc()
    return _NC


LAST_RESULTS = None


def kernel(x: np.ndarray) -> np.ndarray:
    from concourse.bass_utils import run_bass_kernel_spmd

    x = np.asarray(x, dtype=np.float32)
    assert x.shape == (B, 64, 64, 3), x.shape
    # [B,64,64,3] -> per-core [bh, i, b2, j, c] flattened to [2, 64, 768] fp16
    xr = x.reshape(N_CORES, 2, 4, 64, 64, 3).transpose(0, 1, 3, 2, 4, 5)
    xr = np.ascontiguousarray(xr.reshape(N_CORES, 2, 64, 768).astype(np.float16))

    nc = _get_nc()
    tbl = tables()
    in_maps = []
    for c in range(N_CORES):
        m = {"x": xr[c]}
        m.update(tbl)
        in_maps.append(m)

    trace = os.environ.get("BASS_KERNEL_TRACE", "0") == "1"
    res = run_bass_kernel_spmd(
        nc, in_maps, core_ids=list(range(N_CORES)), trace=trace
    )
    global LAST_RESULTS
    LAST_RESULTS = res
    outs = []
    for r in res.results:
        # audio4[g=2bh+s]: [p=(i2,n), r] holding 2*l; image b = 4bh + 2s + i2
        a = np.empty((B_LOC, NS), np.float32)
        for g in range(4):
            bh, s = divmod(g, 2)
            a4 = r[f"audio4_{g}"].astype(np.float32).reshape(2, 64, RMAX)
            for i2 in range(2):
                b = 4 * bh + 2 * s + i2
                a[b, : N * NUM] = a4[i2, :, :NUM].reshape(-1)
                a[b, N * NUM :] = a4[i2, 63, NUM:RMAX]
        a = np.clip(a * 0.5, -32768.0, 32767.0)
        outs.append(a)
    return np.concatenate(outs, axis=0)
